# revision 7
# baseline (speedup 1.0000x reference)
"""LIF neuron scan (T=4) over (4, 32, 128, 56, 56) f32, sharded over 8 NeuronCores.

Per-core shard: 4 batches -> [T=4, P=128, FD=12544] f32. The time scan is
local per element; u = u*0.25 + x_t, spike = u > 1, hard reset.

v3: memory-bound design, ~26.5 MB/core HBM traffic (25.7 MB loads +
0.8 MB bit-packed stores):

- Loads stream on the sync HWDGE queue, [128, 3136] f32 tiles.
- Spikes s = sigmoid(2^30*(u-1)) on the Activation engine, bf16 out.
  The scale/bias path is full f32, so s is exactly 0.0/1.0 whenever
  u != 1.0 exactly (|u-1| >= 2^-24 => |arg| >= 64 => sigmoid saturates).
- The idle Tensor engine bit-packs: psum[64h+m, f] += sum_j 2^(t+4j) *
  s_t[2m+j, f] accumulated over t (bf16 matmul with power-of-2 weights,
  exact integer arithmetic in f32 psum). One byte encodes 4 timesteps x
  2 partition rows -> 8x less store traffic than fp8-per-element.
- psum -> uint8 via Activation-engine Copy, stores on the scalar HWDGE
  queue; host unpacks bits.
- Membrane recurrence: updates u = 0.25*r + x_t are Vector STT; the 12
  hard resets r = (u<=1)*u are split 6/6 between Vector (STT) and
  GpSimd (tensor_scalar mask + tensor_tensor mult; Pool rejects STT),
  balancing both engines at ~62us, under the ~72us HBM load floor.
"""

import numpy as np
import ml_dtypes

import concourse.bass as bass
import concourse.mybir as mybir
import concourse.tile as tile
from concourse.vector_clock import ScopedClock
from concourse.bass_utils import run_bass_kernel_spmd

T, B, C, H, W = 4, 32, 128, 56, 56
NCORES = 8
NPER = B // NCORES            # batches per core
NELEM = NPER * C * H * W      # 1,605,632 elements per core per timestep
P = 128
FD = NELEM // P               # 12544
F = 3136                      # chunk width -> 4 chunks
NCH = FD // F
FS = 448                      # matmul sub-chunk (psum bank: 448*4B <= 2KB)
NSUB = F // FS                # 7
DECAY = 0.25
VTH = 1.0
SCALE = float(2 ** 30)        # sigmoid pre-scale: |u-1|>=2^-24 -> |arg|>=64

_MAXW = 1


def _split_drain_and_barrier(self, tick_clock, wait_clock):
    # This walrus build's CoreV3 setupSyncWait rejects >1 sem wait on a
    # TPB_CTRL (Drain) instruction; spread the tail-drain waits across
    # sequential drains on the same engine (equivalent ordering).
    drain_inst = self.nc.sync.drain()
    wait_clock.add_sem_waits(
        drain_inst.ins, ScopedClock({None: tick_clock.global_clock})
    )
    waits = list(drain_inst.ins.sync_info.on_wait)
    if len(waits) > _MAXW:
        drain_inst.ins.sync_info.on_wait = waits[:_MAXW]
        rest = waits[_MAXW:]
        while rest:
            extra = self.nc.sync.drain()
            si = extra.ins.sync_info
            if si is None:
                extra.ins.sync_info = bass._bass_rust.SyncInfo(
                    on_wait=rest[:_MAXW], on_update=[]
                )
            else:
                si.on_wait = rest[:_MAXW]
            rest = rest[_MAXW:]

    self.nc.all_engine_barrier()
    assert self.sems is not None
    popped = self.nc._tile_sem_poison_stack.pop()
    assert popped is self._sem_poison
    self.nc.clear_and_free_semaphores(list(self.sems.allocated().values()))
    self.nc.all_engine_barrier()


def _install_patch():
    if getattr(tile.TileContext, "_lif_drain_patched", False):
        return
    tile.TileContext._drain_and_barrier = _split_drain_and_barrier
    tile.TileContext._lif_drain_patched = True


def _split_waits(nc, maxw=_MAXW):
    # Generic post-pass for the same walrus limitation: any instruction
    # carrying more than `maxw` sem waits gets the excess peeled onto
    # standalone NOPs inserted immediately before it on the same engine --
    # the engine stalls at the NOPs instead, identical blocking semantics.
    k = 0
    for fn in nc.m.functions:
        for bb in fn.blocks:
            out = []
            for ins in bb.instructions:
                si = getattr(ins, "sync_info", None)
                if si is not None and len(si.on_wait) > maxw:
                    waits = list(si.on_wait)
                    for w in waits[:-maxw] if maxw else waits:
                        k += 1
                        out.append(
                            mybir.InstNoOp(
                                name=f"splitw_{k}_{ins.name}",
                                engine=ins.engine,
                                bass_nofuse=True,
                                sync_info=mybir.SyncInfo(
                                    on_wait=[w], on_update=[]
                                ),
                            )
                        )
                    si.on_wait = waits[-maxw:] if maxw else []
                out.append(ins)
            bb.instructions = out


def _pack_weights():
    # w[2m+j, t*64 + m] = 2^(t + 4j): byte bit (t + 4j) <- spike of
    # partition row 2m+j at timestep t. Powers of two: exact in bf16/f32.
    w = np.zeros((P, T * 64), dtype=np.float32)
    for t in range(T):
        for m in range(64):
            for j in range(2):
                w[2 * m + j, t * 64 + m] = float(2 ** (t + 4 * j))
    return w.astype(ml_dtypes.bfloat16)


def _build(bufs=2):
    _install_patch()
    nc = bass.Bass()
    f32 = mybir.dt.float32
    bf16 = mybir.dt.bfloat16
    u8 = mybir.dt.uint8
    x = nc.dram_tensor("x", [T, P, FD], f32, kind="ExternalInput")
    w = nc.dram_tensor("w", [P, T * 64], bf16, kind="ExternalInput")
    # y[q, 64h+m, sub*448+f] = packed byte of chunk 2q+h, rows (2m, 2m+1)
    y = nc.dram_tensor("y", [NCH // 2, P, F], u8, kind="ExternalOutput")
    LE = mybir.AluOpType.is_le
    MUL, ADD = mybir.AluOpType.mult, mybir.AluOpType.add
    SIG = mybir.ActivationFunctionType.Sigmoid
    CPY = mybir.ActivationFunctionType.Copy

    with tile.TileContext(nc) as tc:
        with tc.tile_pool(name="p", bufs=bufs) as pool, \
             tc.tile_pool(name="wp", bufs=1) as wpool, \
             tc.tile_pool(name="ps", bufs=1, space="PSUM") as psp:
            negb = wpool.tile([P, 1], f32, tag="negb", name="negb")
            nc.gpsimd.memset(negb[:], -SCALE)
            wt = wpool.tile([P, T * 64], bf16, tag="w", name="w")
            nc.sync.dma_start(wt[:], w[:, :])
            psum = [
                psp.tile([P, FS], f32, tag=f"ps{s}", name=f"ps{s}")
                for s in range(NSUB)
            ]
            for g in range(NCH):
                h = g % 2          # partition half within the psum pair
                sl = bass.ts(g, F)
                xt = [
                    pool.tile([P, F], f32, tag=f"x{t}", name=f"x{t}_{g}")
                    for t in range(T)
                ]
                st = [
                    pool.tile([P, F], bf16, tag=f"s{t % 2}", name=f"s{t}_{g}")
                    for t in range(T)
                ]
                rt = pool.tile([P, F], f32, tag="r", name=f"r_{g}")
                mt = pool.tile([P, F], f32, tag="m", name=f"m_{g}")
                for t in range(T):
                    nc.sync.dma_start(xt[t][:], x[t, :, sl])
                for t in range(T):
                    if t > 0:
                        # u_t = 0.25*r_{t-1} + x_t  (in place on x_t)
                        nc.vector.scalar_tensor_tensor(
                            xt[t][:], rt[:], DECAY, xt[t][:], MUL, ADD
                        )
                    # spike: s = sigmoid(2^30*u - 2^30) -> exactly {0, 1}
                    nc.scalar.activation(
                        st[t][:], xt[t][:], SIG, bias=negb[:], scale=SCALE
                    )
                    if t < T - 1:
                        # hard reset into scratch: r = (u <= 1) * u.
                        # 6 of 12 resets on GpSimd (Pool has no STT: mask
                        # then multiply), 6 on Vector (STT), balancing both
                        # engines under the HBM load floor.
                        on_g = t == 0 or (t == 1 and h == 0)
                        if on_g:
                            nc.gpsimd.tensor_scalar(
                                mt[:], xt[t][:], VTH, None, LE
                            )
                            nc.gpsimd.tensor_tensor(
                                rt[:], mt[:], xt[t][:], MUL
                            )
                        else:
                            nc.vector.scalar_tensor_tensor(
                                rt[:], xt[t][:], VTH, xt[t][:], LE, MUL
                            )
                    # bit-pack on the Tensor engine: accumulate over t
                    for s in range(NSUB):
                        nc.tensor.matmul(
                            psum[s][64 * h : 64 * (h + 1), :],
                            wt[:, bass.ts(t, 64)],
                            st[t][:, bass.ts(s, FS)],
                            start=(t == 0),
                            stop=(t == T - 1),
                        )
                if h == 1:
                    # both halves of the pair accumulated: cast + store
                    u8t = pool.tile([P, F], u8, tag="u8", name=f"u8_{g // 2}")
                    for s in range(NSUB):
                        nc.scalar.activation(
                            u8t[:, bass.ts(s, FS)], psum[s][:], CPY
                        )
                    nc.scalar.dma_start(y[g // 2, :, :], u8t[:])
    _split_waits(nc)
    return nc


_cache = {}

_W_HOST = _pack_weights()


def _launch(shards, **kw):
    if "nc" not in _cache:
        _cache["nc"] = _build()
    return run_bass_kernel_spmd(
        _cache["nc"],
        [{"x": s, "w": _W_HOST} for s in shards],
        core_ids=list(range(NCORES)),
        **kw,
    )


def _decode(yq):
    # yq: [NCH//2, 128, 3136] uint8 -> spikes [T, 128, FD] f32.
    bits = np.unpackbits(yq[..., None], axis=-1, bitorder="little")
    # [q, 64h+m, f, 4t+... ] bit index = t + 4j
    arr = bits.reshape(NCH // 2, 2, 64, F, 2, T)   # [q, h, m, f, j, t]
    arr = arr.transpose(5, 2, 4, 0, 1, 3)          # [t, m, j, q, h, f]
    return arr.reshape(T, P, FD).astype(np.float32)


def kernel(x, _launch_kw=None):
    x = np.ascontiguousarray(np.asarray(x, dtype=np.float32))
    assert x.shape == (T, B, C, H, W), x.shape
    shards = [
        np.ascontiguousarray(x[:, i * NPER : (i + 1) * NPER]).reshape(T, P, FD)
        for i in range(NCORES)
    ]
    res = _launch(shards, **(_launch_kw or {}))
    _cache["last_results"] = res
    outs = [
        _decode(np.asarray(r["y"])).reshape(T, NPER, C, H, W)
        for r in res.results
    ]
    return np.concatenate(outs, axis=1)


# revision 9
# speedup vs baseline: 3.7046x; 3.7046x over previous
"""LIF neuron scan (T=4) over (4, 32, 128, 56, 56) f32, sharded over 8 NeuronCores.

Per-core shard: 4 batches -> [T=4, P=128, FD=12544] f32. The time scan is
local per element; u = u*0.25 + x_t, spike = u > 1, hard reset.

v3: memory-bound design, ~26.5 MB/core HBM traffic (25.7 MB loads +
0.8 MB bit-packed stores):

- Loads stream on the sync HWDGE queue, [128, 3136] f32 tiles.
- Spikes s = sigmoid(2^30*(u-1)) on the Activation engine, bf16 out.
  The scale/bias path is full f32, so s is exactly 0.0/1.0 whenever
  u != 1.0 exactly (|u-1| >= 2^-24 => |arg| >= 64 => sigmoid saturates).
- The idle Tensor engine bit-packs: psum[64h+m, f] += sum_j 2^(t+4j) *
  s_t[2m+j, f] accumulated over t (bf16 matmul with power-of-2 weights,
  exact integer arithmetic in f32 psum). One byte encodes 4 timesteps x
  2 partition rows -> 8x less store traffic than fp8-per-element.
- psum -> uint8 via Activation-engine Copy, stores on the scalar HWDGE
  queue; host unpacks bits.
- Membrane recurrence: updates u = 0.25*r + x_t are Vector STT; the 12
  hard resets r = (u<=1)*u are split 6/6 between Vector (STT) and
  GpSimd (tensor_scalar mask + tensor_tensor mult; Pool rejects STT),
  balancing both engines at ~62us, under the ~72us HBM load floor.
"""

import numpy as np
import ml_dtypes

import concourse.bass as bass
import concourse.mybir as mybir
import concourse.tile as tile
from concourse.vector_clock import ScopedClock
from concourse.bass_utils import run_bass_kernel_spmd

T, B, C, H, W = 4, 32, 128, 56, 56
NCORES = 8
NPER = B // NCORES            # batches per core
NELEM = NPER * C * H * W      # 1,605,632 elements per core per timestep
P = 128
FD = NELEM // P               # 12544
F = 3136                      # chunk width -> 4 chunks
NCH = FD // F
FS = 448                      # matmul sub-chunk (psum bank: 448*4B <= 2KB)
NSUB = F // FS                # 7
DECAY = 0.25
VTH = 1.0
SCALE = float(2 ** 30)        # sigmoid pre-scale: |u-1|>=2^-24 -> |arg|>=64

_MAXW = 1


def _split_drain_and_barrier(self, tick_clock, wait_clock):
    # This walrus build's CoreV3 setupSyncWait rejects >1 sem wait on a
    # TPB_CTRL (Drain) instruction; spread the tail-drain waits across
    # sequential drains on the same engine (equivalent ordering).
    drain_inst = self.nc.sync.drain()
    wait_clock.add_sem_waits(
        drain_inst.ins, ScopedClock({None: tick_clock.global_clock})
    )
    waits = list(drain_inst.ins.sync_info.on_wait)
    if len(waits) > _MAXW:
        drain_inst.ins.sync_info.on_wait = waits[:_MAXW]
        rest = waits[_MAXW:]
        while rest:
            extra = self.nc.sync.drain()
            si = extra.ins.sync_info
            if si is None:
                extra.ins.sync_info = bass._bass_rust.SyncInfo(
                    on_wait=rest[:_MAXW], on_update=[]
                )
            else:
                si.on_wait = rest[:_MAXW]
            rest = rest[_MAXW:]

    self.nc.all_engine_barrier()
    assert self.sems is not None
    popped = self.nc._tile_sem_poison_stack.pop()
    assert popped is self._sem_poison
    self.nc.clear_and_free_semaphores(list(self.sems.allocated().values()))
    self.nc.all_engine_barrier()


def _install_patch():
    if getattr(tile.TileContext, "_lif_drain_patched", False):
        return
    tile.TileContext._drain_and_barrier = _split_drain_and_barrier
    tile.TileContext._lif_drain_patched = True


def _split_waits(nc, maxw=_MAXW):
    # Generic post-pass for the same walrus limitation: any instruction
    # carrying more than `maxw` sem waits gets the excess peeled onto
    # standalone NOPs inserted immediately before it on the same engine --
    # the engine stalls at the NOPs instead, identical blocking semantics.
    k = 0
    for fn in nc.m.functions:
        for bb in fn.blocks:
            out = []
            for ins in bb.instructions:
                si = getattr(ins, "sync_info", None)
                if si is not None and len(si.on_wait) > maxw:
                    waits = list(si.on_wait)
                    for w in waits[:-maxw] if maxw else waits:
                        k += 1
                        out.append(
                            mybir.InstNoOp(
                                name=f"splitw_{k}_{ins.name}",
                                engine=ins.engine,
                                bass_nofuse=True,
                                sync_info=mybir.SyncInfo(
                                    on_wait=[w], on_update=[]
                                ),
                            )
                        )
                    si.on_wait = waits[-maxw:] if maxw else []
                out.append(ins)
            bb.instructions = out


def _pack_weights():
    # w[2m+j, t*64 + m] = 2^(t + 4j): byte bit (t + 4j) <- spike of
    # partition row 2m+j at timestep t. Powers of two: exact in bf16/f32.
    w = np.zeros((P, T * 64), dtype=np.float32)
    for t in range(T):
        for m in range(64):
            for j in range(2):
                w[2 * m + j, t * 64 + m] = float(2 ** (t + 4 * j))
    return w.astype(ml_dtypes.bfloat16)


def _build(bufs=2):
    _install_patch()
    nc = bass.Bass()
    f32 = mybir.dt.float32
    bf16 = mybir.dt.bfloat16
    u8 = mybir.dt.uint8
    x = nc.dram_tensor("x", [T, P, FD], f32, kind="ExternalInput")
    w = nc.dram_tensor("w", [P, T * 64], bf16, kind="ExternalInput")
    # y[q, 64h+m, sub*448+f] = packed byte of chunk 2q+h, rows (2m, 2m+1)
    y = nc.dram_tensor("y", [NCH // 2, P, F], u8, kind="ExternalOutput")
    LE = mybir.AluOpType.is_le
    MUL, ADD = mybir.AluOpType.mult, mybir.AluOpType.add
    SIG = mybir.ActivationFunctionType.Sigmoid
    CPY = mybir.ActivationFunctionType.Copy

    with tile.TileContext(nc) as tc:
        with tc.tile_pool(name="p", bufs=bufs) as pool, \
             tc.tile_pool(name="wp", bufs=1) as wpool, \
             tc.tile_pool(name="ps", bufs=1, space="PSUM") as psp:
            negb = wpool.tile([P, 1], f32, tag="negb", name="negb")
            nc.gpsimd.memset(negb[:], -SCALE)
            wt = wpool.tile([P, T * 64], bf16, tag="w", name="w")
            nc.sync.dma_start(wt[:], w[:, :])
            psum = [
                psp.tile([P, FS], f32, tag=f"ps{s}", name=f"ps{s}")
                for s in range(NSUB)
            ]
            for g in range(NCH):
                h = g % 2          # partition half within the psum pair
                sl = bass.ts(g, F)
                xt = [
                    pool.tile([P, F], f32, tag=f"x{t}", name=f"x{t}_{g}")
                    for t in range(T)
                ]
                st = [
                    pool.tile([P, F], bf16, tag=f"s{t % 2}", name=f"s{t}_{g}")
                    for t in range(T)
                ]
                rt = pool.tile([P, F], f32, tag="r", name=f"r_{g}")
                for t in range(T):
                    nc.sync.dma_start(xt[t][:], x[t, :, sl])
                for t in range(T):
                    if t > 0:
                        # u_t = 0.25*r_{t-1} + x_t  (in place on x_t)
                        nc.vector.scalar_tensor_tensor(
                            xt[t][:], rt[:], DECAY, xt[t][:], MUL, ADD
                        )
                    # spike: s = sigmoid(2^30*u - 2^30) -> exactly {0, 1}
                    nc.scalar.activation(
                        st[t][:], xt[t][:], SIG, bias=negb[:], scale=SCALE
                    )
                    if t < T - 1:
                        # hard reset into scratch: r = (u <= 1) * u.
                        # All resets stay on Vector: GpSimd tensor ops
                        # measured 8-48us per tile AND their SBUF-port
                        # contention slows concurrent DVE STTs 2-4x.
                        nc.vector.scalar_tensor_tensor(
                            rt[:], xt[t][:], VTH, xt[t][:], LE, MUL
                        )
                    # bit-pack on the Tensor engine: accumulate over t
                    for s in range(NSUB):
                        nc.tensor.matmul(
                            psum[s][64 * h : 64 * (h + 1), :],
                            wt[:, bass.ts(t, 64)],
                            st[t][:, bass.ts(s, FS)],
                            start=(t == 0),
                            stop=(t == T - 1),
                        )
                if h == 1:
                    # both halves of the pair accumulated: cast + store
                    u8t = pool.tile([P, F], u8, tag="u8", name=f"u8_{g // 2}")
                    for s in range(NSUB):
                        nc.scalar.activation(
                            u8t[:, bass.ts(s, FS)], psum[s][:], CPY
                        )
                    nc.scalar.dma_start(y[g // 2, :, :], u8t[:])
    _split_waits(nc)
    return nc


_cache = {}

_W_HOST = _pack_weights()


def _launch(shards, **kw):
    if "nc" not in _cache:
        _cache["nc"] = _build()
    return run_bass_kernel_spmd(
        _cache["nc"],
        [{"x": s, "w": _W_HOST} for s in shards],
        core_ids=list(range(NCORES)),
        **kw,
    )


def _decode(yq):
    # yq: [NCH//2, 128, 3136] uint8 -> spikes [T, 128, FD] f32.
    bits = np.unpackbits(yq[..., None], axis=-1, bitorder="little")
    # [q, 64h+m, f, 4t+... ] bit index = t + 4j
    arr = bits.reshape(NCH // 2, 2, 64, F, 2, T)   # [q, h, m, f, j, t]
    arr = arr.transpose(5, 2, 4, 0, 1, 3)          # [t, m, j, q, h, f]
    return arr.reshape(T, P, FD).astype(np.float32)


def kernel(x, _launch_kw=None):
    x = np.ascontiguousarray(np.asarray(x, dtype=np.float32))
    assert x.shape == (T, B, C, H, W), x.shape
    shards = [
        np.ascontiguousarray(x[:, i * NPER : (i + 1) * NPER]).reshape(T, P, FD)
        for i in range(NCORES)
    ]
    res = _launch(shards, **(_launch_kw or {}))
    _cache["last_results"] = res
    outs = [
        _decode(np.asarray(r["y"])).reshape(T, NPER, C, H, W)
        for r in res.results
    ]
    return np.concatenate(outs, axis=1)


# revision 11
# speedup vs baseline: 3.7766x; 1.0194x over previous
"""LIF neuron scan (T=4) over (4, 32, 128, 56, 56) f32, sharded over 8 NeuronCores.

Per-core shard: 4 batches -> [T=4, P=128, FD=12544] f32. The time scan is
local per element; u = u*0.25 + x_t, spike = u > 1, hard reset.

v6: memory-bound design, ~26.5 MB/core HBM traffic (25.7 MB f32 loads +
0.8 MB bit-packed stores):

- Loads stream on the sync HWDGE queue ([128, 3136] f32 tiles); the
  pack weights ride the ACT HWDGE ring so they don't delay x loads.
- Membrane recurrence (12 update STTs u = 0.25*r + x_t, 12 reset STTs
  r = (u<=1)*u) runs entirely on the Vector engine (~82us busy, the
  critical path). GpSimd tensor ops measured 8-48us/tile and their
  SBUF-port contention halves DVE throughput; fp16 operands neither
  engage a 2x DVE mode for STT nor keep full compare precision.
- Spikes s = sigmoid(2^30*(u-1)) on the Activation engine, bf16 out
  (exact {0,1} for u != 1: |u-1| >= 2^-24 => |arg| >= 64 saturates; the
  activation scale/bias path is full f32). The last chunk's final
  compare runs on Vector instead (is_gt, engine-local) to cut the tail.
- The idle Tensor engine bit-packs: psum[64h+m, f] += sum_j 2^(t+4j) *
  s_t[2m+j, f] accumulated over t (bf16 matmul with power-of-2 weights,
  exact integer arithmetic in f32 psum). One byte encodes 4 timesteps x
  2 partition rows -> 8x less store traffic than one byte per element.
- psum -> uint8 casts on the Activation engine (split with Vector for
  the final pair), stores on the scalar HWDGE queue; host unpacks bits.
- Head trim: the first chunk's t=0 load and reset are split in quarters
  so Vector starts as soon as the first 0.4 MB lands.
"""

import numpy as np
import ml_dtypes

import concourse.bass as bass
import concourse.mybir as mybir
import concourse.tile as tile
from concourse.vector_clock import ScopedClock
from concourse.bass_utils import run_bass_kernel_spmd

T, B, C, H, W = 4, 32, 128, 56, 56
NCORES = 8
NPER = B // NCORES            # batches per core
NELEM = NPER * C * H * W      # 1,605,632 elements per core per timestep
P = 128
FD = NELEM // P               # 12544
F = 3136                      # chunk width -> 4 chunks
NCH = FD // F
FS = 448                      # matmul sub-chunk (psum bank: 448*4B <= 2KB)
NSUB = F // FS                # 7
DECAY = 0.25
VTH = 1.0
SCALE = float(2 ** 30)        # sigmoid pre-scale: |u-1|>=2^-24 -> |arg|>=64

_MAXW = 1


def _split_drain_and_barrier(self, tick_clock, wait_clock):
    # This walrus build's CoreV3 setupSyncWait rejects >1 sem wait on a
    # TPB_CTRL (Drain) instruction; spread the tail-drain waits across
    # sequential drains on the same engine (equivalent ordering).
    drain_inst = self.nc.sync.drain()
    wait_clock.add_sem_waits(
        drain_inst.ins, ScopedClock({None: tick_clock.global_clock})
    )
    waits = list(drain_inst.ins.sync_info.on_wait)
    if len(waits) > _MAXW:
        drain_inst.ins.sync_info.on_wait = waits[:_MAXW]
        rest = waits[_MAXW:]
        while rest:
            extra = self.nc.sync.drain()
            si = extra.ins.sync_info
            if si is None:
                extra.ins.sync_info = bass._bass_rust.SyncInfo(
                    on_wait=rest[:_MAXW], on_update=[]
                )
            else:
                si.on_wait = rest[:_MAXW]
            rest = rest[_MAXW:]

    self.nc.all_engine_barrier()
    assert self.sems is not None
    popped = self.nc._tile_sem_poison_stack.pop()
    assert popped is self._sem_poison
    self.nc.clear_and_free_semaphores(list(self.sems.allocated().values()))
    self.nc.all_engine_barrier()


def _install_patch():
    if getattr(tile.TileContext, "_lif_drain_patched", False):
        return
    tile.TileContext._drain_and_barrier = _split_drain_and_barrier
    tile.TileContext._lif_drain_patched = True


def _split_waits(nc, maxw=_MAXW):
    # Generic post-pass for the same walrus limitation: any instruction
    # carrying more than `maxw` sem waits gets the excess peeled onto
    # standalone NOPs inserted immediately before it on the same engine --
    # the engine stalls at the NOPs instead, identical blocking semantics.
    k = 0
    for fn in nc.m.functions:
        for bb in fn.blocks:
            out = []
            for ins in bb.instructions:
                si = getattr(ins, "sync_info", None)
                if si is not None and len(si.on_wait) > maxw:
                    waits = list(si.on_wait)
                    for w in waits[:-maxw] if maxw else waits:
                        k += 1
                        out.append(
                            mybir.InstNoOp(
                                name=f"splitw_{k}_{ins.name}",
                                engine=ins.engine,
                                bass_nofuse=True,
                                sync_info=mybir.SyncInfo(
                                    on_wait=[w], on_update=[]
                                ),
                            )
                        )
                    si.on_wait = waits[-maxw:] if maxw else []
                out.append(ins)
            bb.instructions = out


def _pack_weights():
    # w[2m+j, t*64 + m] = 2^(t + 4j): byte bit (t + 4j) <- spike of
    # partition row 2m+j at timestep t. Powers of two: exact in bf16/f32.
    w = np.zeros((P, T * 64), dtype=np.float32)
    for t in range(T):
        for m in range(64):
            for j in range(2):
                w[2 * m + j, t * 64 + m] = float(2 ** (t + 4 * j))
    return w.astype(ml_dtypes.bfloat16)


def _build(bufs=2):
    _install_patch()
    nc = bass.Bass()
    f32 = mybir.dt.float32
    bf16 = mybir.dt.bfloat16
    u8 = mybir.dt.uint8
    x = nc.dram_tensor("x", [T, P, FD], f32, kind="ExternalInput")
    w = nc.dram_tensor("w", [P, T * 64], bf16, kind="ExternalInput")
    # y[q, 64h+m, sub*448+f] = packed byte of chunk 2q+h, rows (2m, 2m+1)
    y = nc.dram_tensor("y", [NCH // 2, P, F], u8, kind="ExternalOutput")
    LE, GT = mybir.AluOpType.is_le, mybir.AluOpType.is_gt
    MUL, ADD = mybir.AluOpType.mult, mybir.AluOpType.add
    SIG = mybir.ActivationFunctionType.Sigmoid
    CPY = mybir.ActivationFunctionType.Copy

    with tile.TileContext(nc) as tc:
        with tc.tile_pool(name="p", bufs=bufs) as pool, \
             tc.tile_pool(name="wp", bufs=1) as wpool, \
             tc.tile_pool(name="ps", bufs=1, space="PSUM") as psp:
            negb = wpool.tile([P, 1], f32, tag="negb", name="negb")
            nc.gpsimd.memset(negb[:], -SCALE)
            wt = wpool.tile([P, T * 64], bf16, tag="w", name="w")
            # w rides the ACT HWDGE ring so it does not delay x0 loads
            nc.scalar.dma_start(wt[:], w[:, :])
            psum = [
                psp.tile([P, FS], f32, tag=f"ps{s}", name=f"ps{s}")
                for s in range(NSUB)
            ]
            for g in range(NCH):
                h = g % 2          # partition half within the psum pair
                last = g == NCH - 1
                sl = bass.ts(g, F)
                xt = [
                    pool.tile([P, F], f32, tag=f"x{t}", name=f"x{t}_{g}")
                    for t in range(T)
                ]
                st = [
                    pool.tile([P, F], bf16, tag=f"s{t % 2}", name=f"s{t}_{g}")
                    for t in range(T)
                ]
                rt = pool.tile([P, F], f32, tag="r", name=f"r_{g}")
                if g == 0:
                    # split the very first load in quarters so Vector can
                    # start the t=0 reset as soon as 0.4 MB has landed
                    q4 = F // 4
                    for k in range(4):
                        qsl = slice(k * q4, (k + 1) * q4)
                        nc.sync.dma_start(xt[0][:, qsl], x[0, :, qsl])
                else:
                    nc.sync.dma_start(xt[0][:], x[0, :, sl])
                for t in range(1, T):
                    nc.sync.dma_start(xt[t][:], x[t, :, sl])
                for t in range(T):
                    if t > 0:
                        # u_t = 0.25*r_{t-1} + x_t  (in place on x_t)
                        nc.vector.scalar_tensor_tensor(
                            xt[t][:], rt[:], DECAY, xt[t][:], MUL, ADD
                        )
                    # spike: s = {0,1}. Scalar sigmoid normally; the last
                    # chunk's final compare stays on Vector (idle by then)
                    if last and t == T - 1:
                        nc.vector.tensor_scalar(
                            st[t][:], xt[t][:], VTH, None, GT
                        )
                    else:
                        nc.scalar.activation(
                            st[t][:], xt[t][:], SIG, bias=negb[:], scale=SCALE
                        )
                    if t < T - 1:
                        # hard reset into scratch: r = (u <= 1) * u
                        if g == 0 and t == 0:
                            q4 = F // 4
                            for k in range(4):
                                qsl = slice(k * q4, (k + 1) * q4)
                                nc.vector.scalar_tensor_tensor(
                                    rt[:, qsl], xt[0][:, qsl],
                                    VTH, xt[0][:, qsl], LE, MUL,
                                )
                        else:
                            nc.vector.scalar_tensor_tensor(
                                rt[:], xt[t][:], VTH, xt[t][:], LE, MUL
                            )
                    # bit-pack on the Tensor engine: accumulate over t
                    for s in range(NSUB):
                        nc.tensor.matmul(
                            psum[s][64 * h : 64 * (h + 1), :],
                            wt[:, bass.ts(t, 64)],
                            st[t][:, bass.ts(s, FS)],
                            start=(t == 0),
                            stop=(t == T - 1),
                        )
                if h == 1:
                    # both halves of the pair accumulated: cast + store.
                    # Final pair: split converts Vector/Scalar and store in
                    # two halves to shorten the tail.
                    u8t = pool.tile([P, F], u8, tag="u8", name=f"u8_{g // 2}")
                    for s in range(NSUB):
                        if last and s % 2 == 1:
                            nc.vector.tensor_copy(
                                u8t[:, bass.ts(s, FS)], psum[s][:]
                            )
                        else:
                            nc.scalar.activation(
                                u8t[:, bass.ts(s, FS)], psum[s][:], CPY
                            )
                        if last and s == 3:
                            nc.scalar.dma_start(
                                y[g // 2, :, : 4 * FS], u8t[:, : 4 * FS]
                            )
                    if last:
                        nc.scalar.dma_start(
                            y[g // 2, :, 4 * FS :], u8t[:, 4 * FS :]
                        )
                    else:
                        nc.scalar.dma_start(y[g // 2, :, :], u8t[:])
    _split_waits(nc)
    return nc


_cache = {}

_W_HOST = _pack_weights()


def _launch(shards, **kw):
    if "nc" not in _cache:
        _cache["nc"] = _build()
    return run_bass_kernel_spmd(
        _cache["nc"],
        [{"x": s, "w": _W_HOST} for s in shards],
        core_ids=list(range(NCORES)),
        **kw,
    )


def _decode(yq):
    # yq: [NCH//2, 128, 3136] uint8 -> spikes [T, 128, FD] f32.
    bits = np.unpackbits(yq[..., None], axis=-1, bitorder="little")
    arr = bits.reshape(NCH // 2, 2, 64, F, 2, T)   # [q, h, m, f, j, t]
    arr = arr.transpose(5, 2, 4, 0, 1, 3)          # [t, m, j, q, h, f]
    return arr.reshape(T, P, FD).astype(np.float32)


def kernel(x, _launch_kw=None):
    x = np.ascontiguousarray(np.asarray(x, dtype=np.float32))
    assert x.shape == (T, B, C, H, W), x.shape
    shards = [
        np.ascontiguousarray(x[:, i * NPER : (i + 1) * NPER]).reshape(T, P, FD)
        for i in range(NCORES)
    ]
    res = _launch(shards, **(_launch_kw or {}))
    _cache["last_results"] = res
    outs = [
        _decode(np.asarray(r["y"])).reshape(T, NPER, C, H, W)
        for r in res.results
    ]
    return np.concatenate(outs, axis=1)


# revision 13
# speedup vs baseline: 3.8498x; 1.0194x over previous
"""LIF neuron scan (T=4) over (4, 32, 128, 56, 56) f32, sharded over 8 NeuronCores.

Per-core shard: 4 batches -> [T=4, P=128, FD=12544] f32. The time scan is
local per element; u = u*0.25 + x_t, spike = u > 1, hard reset.

v2: memory-bound design. Loads (25.7 MB/core f32) stream on the sync
HWDGE queue; spikes are written as 1-byte fp8e5 "relu encodings"
e = relu(2^20*(u-1)) emitted by the Activation engine (e > 0 <=> u > 1,
exactly: the 2^20 scale keeps every representable positive far above
the fp8e5 subnormal range), stores on the scalar HWDGE queue. The
Vector engine carries the 6 recurrence ops per chunk (3 membrane
updates + 3 resets, all f32 STT); resets write a scratch tile so the
Activation compares never block the Vector chain. Host decodes
spikes = (e > 0).
"""

import numpy as np

import concourse.bass as bass
import concourse.mybir as mybir
import concourse.tile as tile
from concourse.vector_clock import ScopedClock
from concourse.bass_utils import run_bass_kernel_spmd

T, B, C, H, W = 4, 32, 128, 56, 56
NCORES = 8
NPER = B // NCORES            # batches per core
NELEM = NPER * C * H * W      # 1,605,632 elements per core per timestep
P = 128
FD = NELEM // P               # 12544
F = 3136                      # chunk width -> 4 chunks
NCH = FD // F
DECAY = 0.25
VTH = 1.0
SCALE = float(2 ** 20)        # relu pre-scale: keeps positives >= 0.125

_MAXW = 1


def _split_drain_and_barrier(self, tick_clock, wait_clock):
    # This walrus build's CoreV3 setupSyncWait rejects >1 sem wait on a
    # TPB_CTRL (Drain) instruction; spread the tail-drain waits across
    # sequential drains on the same engine (equivalent ordering).
    drain_inst = self.nc.sync.drain()
    wait_clock.add_sem_waits(
        drain_inst.ins, ScopedClock({None: tick_clock.global_clock})
    )
    waits = list(drain_inst.ins.sync_info.on_wait)
    if len(waits) > _MAXW:
        drain_inst.ins.sync_info.on_wait = waits[:_MAXW]
        rest = waits[_MAXW:]
        while rest:
            extra = self.nc.sync.drain()
            si = extra.ins.sync_info
            if si is None:
                extra.ins.sync_info = bass._bass_rust.SyncInfo(
                    on_wait=rest[:_MAXW], on_update=[]
                )
            else:
                si.on_wait = rest[:_MAXW]
            rest = rest[_MAXW:]

    self.nc.all_engine_barrier()
    assert self.sems is not None
    popped = self.nc._tile_sem_poison_stack.pop()
    assert popped is self._sem_poison
    self.nc.clear_and_free_semaphores(list(self.sems.allocated().values()))
    self.nc.all_engine_barrier()


def _install_patch():
    if getattr(tile.TileContext, "_lif_drain_patched", False):
        return
    tile.TileContext._drain_and_barrier = _split_drain_and_barrier
    tile.TileContext._lif_drain_patched = True


def _split_waits(nc, maxw=_MAXW):
    # Generic post-pass for the same walrus limitation: any instruction
    # carrying more than `maxw` sem waits gets the excess peeled onto
    # standalone NOPs inserted immediately before it on the same engine --
    # the engine stalls at the NOPs instead, identical blocking semantics.
    k = 0
    for fn in nc.m.functions:
        for bb in fn.blocks:
            out = []
            for ins in bb.instructions:
                si = getattr(ins, "sync_info", None)
                if si is not None and len(si.on_wait) > maxw:
                    waits = list(si.on_wait)
                    for w in waits[:-maxw] if maxw else waits:
                        k += 1
                        out.append(
                            mybir.InstNoOp(
                                name=f"splitw_{k}_{ins.name}",
                                engine=ins.engine,
                                bass_nofuse=True,
                                sync_info=mybir.SyncInfo(
                                    on_wait=[w], on_update=[]
                                ),
                            )
                        )
                    si.on_wait = waits[-maxw:] if maxw else []
                out.append(ins)
            bb.instructions = out


def _build(f=F, bufs=2):
    _install_patch()
    nch = FD // f
    nc = bass.Bass()
    x = nc.dram_tensor("x", [T, P, FD], mybir.dt.float32, kind="ExternalInput")
    y = nc.dram_tensor("y", [T, P, FD], mybir.dt.float8e5, kind="ExternalOutput")
    f32 = mybir.dt.float32
    fp8 = mybir.dt.float8e5
    LE = mybir.AluOpType.is_le
    MUL, ADD = mybir.AluOpType.mult, mybir.AluOpType.add
    RELU = mybir.ActivationFunctionType.Relu

    with tile.TileContext(nc) as tc:
        with tc.tile_pool(name="p", bufs=bufs) as pool:
            neg = pool.tile([P, 1], f32, tag="neg", name="neg")
            nc.gpsimd.memset(neg[:], -SCALE)
            for g in range(nch):
                sl = bass.ts(g, f)
                xt = [
                    pool.tile([P, f], f32, tag=f"x{t}", name=f"x{t}_{g}")
                    for t in range(T)
                ]
                st = [
                    pool.tile([P, f], fp8, tag=f"s{t}", name=f"s{t}_{g}")
                    for t in range(T)
                ]
                rt = pool.tile([P, f], f32, tag="r", name=f"r_{g}")
                if g == 0:
                    # head trim: split the very first load in quarters so
                    # the Vector engine starts the t=0 reset as soon as
                    # the first 0.4 MB lands instead of waiting ~4us for
                    # the whole tile.
                    q4 = f // 4
                    for k in range(4):
                        qsl = slice(k * q4, (k + 1) * q4)
                        nc.sync.dma_start(xt[0][:, qsl], x[0, :, qsl])
                else:
                    nc.sync.dma_start(xt[0][:], x[0, :, sl])
                for t in range(1, T):
                    nc.sync.dma_start(xt[t][:], x[t, :, sl])
                for t in range(T):
                    if t > 0:
                        # u_t = 0.25*r_{t-1} + x_t  (in place on x_t)
                        nc.vector.scalar_tensor_tensor(
                            xt[t][:], rt[:], DECAY, xt[t][:], MUL, ADD
                        )
                    # spike encoding: e = relu(2^20*u - 2^20); e>0 <=> u>1
                    nc.scalar.activation(
                        st[t][:], xt[t][:], RELU, bias=neg[:], scale=SCALE
                    )
                    if t < T - 1:
                        # hard reset into scratch: r = (u <= 1) * u
                        if g == 0 and t == 0:
                            q4 = f // 4
                            for k in range(4):
                                qsl = slice(k * q4, (k + 1) * q4)
                                nc.vector.scalar_tensor_tensor(
                                    rt[:, qsl], xt[0][:, qsl],
                                    VTH, xt[0][:, qsl], LE, MUL,
                                )
                        else:
                            nc.vector.scalar_tensor_tensor(
                                rt[:], xt[t][:], VTH, xt[t][:], LE, MUL
                            )
                    nc.scalar.dma_start(y[t, :, sl], st[t][:])
    _split_waits(nc)
    return nc


_cache = {}


def _launch(shards, **kw):
    if "nc" not in _cache:
        _cache["nc"] = _build()
    return run_bass_kernel_spmd(
        _cache["nc"],
        [{"x": s} for s in shards],
        core_ids=list(range(NCORES)),
        **kw,
    )


def kernel(x, _launch_kw=None):
    x = np.ascontiguousarray(np.asarray(x, dtype=np.float32))
    assert x.shape == (T, B, C, H, W), x.shape
    shards = [
        np.ascontiguousarray(x[:, i * NPER : (i + 1) * NPER]).reshape(T, P, FD)
        for i in range(NCORES)
    ]
    res = _launch(shards, **(_launch_kw or {}))
    _cache["last_results"] = res
    outs = [
        (np.asarray(r["y"]).astype(np.float32) > 0)
        .astype(np.float32)
        .reshape(T, NPER, C, H, W)
        for r in res.results
    ]
    return np.concatenate(outs, axis=1)



# revision 15
# speedup vs baseline: 3.8717x; 1.0057x over previous
"""LIF neuron scan (T=4) over (4, 32, 128, 56, 56) f32, sharded over 8 NeuronCores.

Per-core shard: 4 batches -> [T=4, P=128, FD=12544] f32. The time scan is
local per element; u = u*0.25 + x_t, spike = u > 1, hard reset.

v2: memory-bound design. Loads (25.7 MB/core f32) stream on the sync
HWDGE queue; spikes are written as 1-byte fp8e5 "relu encodings"
e = relu(2^20*(u-1)) emitted by the Activation engine (e > 0 <=> u > 1,
exactly: the 2^20 scale keeps every representable positive far above
the fp8e5 subnormal range), stores on the scalar HWDGE queue. The
Vector engine carries the 6 recurrence ops per chunk (3 membrane
updates + 3 resets, all f32 STT); resets write a scratch tile so the
Activation compares never block the Vector chain. Host decodes
spikes = (e > 0).
"""

import numpy as np

import concourse.bass as bass
import concourse.mybir as mybir
import concourse.tile as tile
from concourse.vector_clock import ScopedClock
from concourse.bass_utils import run_bass_kernel_spmd

T, B, C, H, W = 4, 32, 128, 56, 56
NCORES = 8
NPER = B // NCORES            # batches per core
NELEM = NPER * C * H * W      # 1,605,632 elements per core per timestep
P = 128
FD = NELEM // P               # 12544
F = 3136                      # chunk width -> 4 chunks
NCH = FD // F
DECAY = 0.25
VTH = 1.0
SCALE = float(2 ** 20)        # relu pre-scale: keeps positives >= 0.125

_MAXW = 1


def _split_drain_and_barrier(self, tick_clock, wait_clock):
    # This walrus build's CoreV3 setupSyncWait rejects >1 sem wait on a
    # TPB_CTRL (Drain) instruction; spread the tail-drain waits across
    # sequential drains on the same engine (equivalent ordering).
    drain_inst = self.nc.sync.drain()
    wait_clock.add_sem_waits(
        drain_inst.ins, ScopedClock({None: tick_clock.global_clock})
    )
    waits = list(drain_inst.ins.sync_info.on_wait)
    if len(waits) > _MAXW:
        drain_inst.ins.sync_info.on_wait = waits[:_MAXW]
        rest = waits[_MAXW:]
        while rest:
            extra = self.nc.sync.drain()
            si = extra.ins.sync_info
            if si is None:
                extra.ins.sync_info = bass._bass_rust.SyncInfo(
                    on_wait=rest[:_MAXW], on_update=[]
                )
            else:
                si.on_wait = rest[:_MAXW]
            rest = rest[_MAXW:]

    self.nc.all_engine_barrier()
    assert self.sems is not None
    popped = self.nc._tile_sem_poison_stack.pop()
    assert popped is self._sem_poison
    self.nc.clear_and_free_semaphores(list(self.sems.allocated().values()))
    self.nc.all_engine_barrier()


def _install_patch():
    if getattr(tile.TileContext, "_lif_drain_patched", False):
        return
    tile.TileContext._drain_and_barrier = _split_drain_and_barrier
    tile.TileContext._lif_drain_patched = True


def _split_waits(nc, maxw=_MAXW):
    # Generic post-pass for the same walrus limitation: any instruction
    # carrying more than `maxw` sem waits gets the excess peeled onto
    # standalone NOPs inserted immediately before it on the same engine --
    # the engine stalls at the NOPs instead, identical blocking semantics.
    k = 0
    for fn in nc.m.functions:
        for bb in fn.blocks:
            out = []
            for ins in bb.instructions:
                si = getattr(ins, "sync_info", None)
                if si is not None and len(si.on_wait) > maxw:
                    waits = list(si.on_wait)
                    for w in waits[:-maxw] if maxw else waits:
                        k += 1
                        out.append(
                            mybir.InstNoOp(
                                name=f"splitw_{k}_{ins.name}",
                                engine=ins.engine,
                                bass_nofuse=True,
                                sync_info=mybir.SyncInfo(
                                    on_wait=[w], on_update=[]
                                ),
                            )
                        )
                    si.on_wait = waits[-maxw:] if maxw else []
                out.append(ins)
            bb.instructions = out


def _build(f=F, bufs=2):
    _install_patch()
    nch = FD // f
    nc = bass.Bass()
    x = nc.dram_tensor("x", [T, P, FD], mybir.dt.float32, kind="ExternalInput")
    y = nc.dram_tensor("y", [T, P, FD], mybir.dt.float8e5, kind="ExternalOutput")
    f32 = mybir.dt.float32
    fp8 = mybir.dt.float8e5
    LE = mybir.AluOpType.is_le
    MUL, ADD = mybir.AluOpType.mult, mybir.AluOpType.add
    RELU = mybir.ActivationFunctionType.Relu

    with tile.TileContext(nc) as tc:
        with tc.tile_pool(name="p", bufs=bufs) as pool:
            neg = pool.tile([P, 1], f32, tag="neg", name="neg")
            nc.gpsimd.memset(neg[:], -SCALE)
            for g in range(nch):
                sl = bass.ts(g, f)
                xt = [
                    pool.tile([P, f], f32, tag=f"x{t}", name=f"x{t}_{g}")
                    for t in range(T)
                ]
                st = [
                    pool.tile([P, f], fp8, tag=f"s{t}", name=f"s{t}_{g}")
                    for t in range(T)
                ]
                rt = pool.tile([P, f], f32, tag="r", name=f"r_{g}")
                if g == 0:
                    # head trim: split the very first load in quarters so
                    # the Vector engine starts the t=0 reset as soon as
                    # the first 0.4 MB lands instead of waiting ~4us for
                    # the whole tile.
                    q4 = f // 4
                    for k in range(4):
                        qsl = slice(k * q4, (k + 1) * q4)
                        nc.sync.dma_start(xt[0][:, qsl], x[0, :, qsl])
                else:
                    nc.sync.dma_start(xt[0][:], x[0, :, sl])
                for t in range(1, T):
                    nc.sync.dma_start(xt[t][:], x[t, :, sl])
                for t in range(T):
                    if t > 0:
                        # u_t = 0.25*r_{t-1} + x_t  (in place on x_t)
                        nc.vector.scalar_tensor_tensor(
                            xt[t][:], rt[:], DECAY, xt[t][:], MUL, ADD
                        )
                    # spike encoding: e = relu(2^20*u - 2^20); e>0 <=> u>1.
                    # Tail trim: the last chunk's final encode+store is
                    # split in halves so the store of the first half
                    # overlaps the encode of the second.
                    if g == nch - 1 and t == T - 1:
                        hf = f // 2
                        for k in range(2):
                            hsl = slice(k * hf, (k + 1) * hf)
                            nc.scalar.activation(
                                st[t][:, hsl], xt[t][:, hsl],
                                RELU, bias=neg[:], scale=SCALE,
                            )
                            nc.scalar.dma_start(
                                y[t, :, g * f + k * hf : g * f + (k + 1) * hf],
                                st[t][:, hsl],
                            )
                    else:
                        nc.scalar.activation(
                            st[t][:], xt[t][:], RELU, bias=neg[:], scale=SCALE
                        )
                    if t < T - 1:
                        # hard reset into scratch: r = (u <= 1) * u
                        if g == 0 and t == 0:
                            q4 = f // 4
                            for k in range(4):
                                qsl = slice(k * q4, (k + 1) * q4)
                                nc.vector.scalar_tensor_tensor(
                                    rt[:, qsl], xt[0][:, qsl],
                                    VTH, xt[0][:, qsl], LE, MUL,
                                )
                        else:
                            nc.vector.scalar_tensor_tensor(
                                rt[:], xt[t][:], VTH, xt[t][:], LE, MUL
                            )
                    if not (g == nch - 1 and t == T - 1):
                        nc.scalar.dma_start(y[t, :, sl], st[t][:])
    _split_waits(nc)
    return nc


_cache = {}


def _launch(shards, **kw):
    if "nc" not in _cache:
        _cache["nc"] = _build()
    return run_bass_kernel_spmd(
        _cache["nc"],
        [{"x": s} for s in shards],
        core_ids=list(range(NCORES)),
        **kw,
    )


def kernel(x, _launch_kw=None):
    x = np.ascontiguousarray(np.asarray(x, dtype=np.float32))
    assert x.shape == (T, B, C, H, W), x.shape
    shards = [
        np.ascontiguousarray(x[:, i * NPER : (i + 1) * NPER]).reshape(T, P, FD)
        for i in range(NCORES)
    ]
    res = _launch(shards, **(_launch_kw or {}))
    _cache["last_results"] = res
    outs = [
        (np.asarray(r["y"]).astype(np.float32) > 0)
        .astype(np.float32)
        .reshape(T, NPER, C, H, W)
        for r in res.results
    ]
    return np.concatenate(outs, axis=1)

